# revision 1
# baseline (speedup 1.0000x reference)
"""Trainium2 Bass kernel for nn_CaptioningRNN (attention-LSTM over T=128 steps).

Sharding: tensor-parallel over the 4H gate dimension across 8 NeuronCores.
Core j owns H-slice j (128 h-rows) of each of the 4 gates (gate columns
{g*1024 + j*128 + i}), so the per-step LSTM cell state (c, h) for that
slice lives entirely on core j. Per step:
  - replicated softmax over the 16 attention scores,
  - local attention slice (DVE mul+reduce against the A slice),
  - AllGather of the bf16 attn^T slices,
  - 16-ktile PE matmul [h; attn] @ [Wh; Wattn] gate-slice (bf16, f32 psum),
  - gates + c/h update (ACT/DVE, f32),
  - PE transpose of h, scores-cross on PE + masked diagonal reduce,
  - AllGather of (hT slice | score partials) in bf16.
x @ Wx + b is precomputed on-device into DRAM in a parallel phase 0.
Host side does layout-only prep (slicing/transposes/casts) and the final
output assembly out[:, :, Hj] <- core j.
"""
import numpy as np
import ml_dtypes

import bass_rust
import concourse.bass as bass
import concourse.mybir as mybir
from concourse import tile
from concourse.alu_op_type import AluOpType
from concourse.bass_utils import run_bass_kernel_spmd

BF16 = ml_dtypes.bfloat16
F32 = mybir.dt.float32
BF = mybir.dt.bfloat16
AF = mybir.ActivationFunctionType
AX = mybir.AxisListType

N, T, D, H, L, R = 64, 128, 512, 1024, 16, 8
HS, GS = H // R, 4 * H // R  # 128, 512
SCALE = 1.0 / np.sqrt(H)


def _split_waits(nc, cap=1):
    """Walrus here rejects >cap sync waits per instruction; hoist extras
    onto preceding same-engine NOPs."""
    ctr = 0
    for fn in nc.m.functions:
        for bb in fn.blocks:
            out, changed = [], False
            for ins in bb.instructions:
                si = ins.sync_info
                if si is not None and si.on_wait and len(si.on_wait) > cap:
                    waits = list(si.on_wait)
                    extra, keep = waits[:-cap], waits[-cap:]
                    for i in range(0, len(extra), cap):
                        out.append(bass_rust.InstNoOp(
                            name=f"zz_waitsplit_{ctr}", engine=ins.engine,
                            sync_info=bass_rust.SyncInfo(
                                on_wait=extra[i:i + cap], on_update=[])))
                        ctr += 1
                    ins.sync_info = bass_rust.SyncInfo(
                        on_wait=keep, on_update=list(si.on_update or []))
                    changed = True
                out.append(ins)
            if changed:
                bb.instructions = out
    return ctr


def _prep_inputs(x, A, Wx, Wh, Wattn, b):
    x = np.asarray(x, np.float32)
    A_flat = np.asarray(A, np.float32).reshape(N, H, L)
    Wx = np.asarray(Wx, np.float32)
    Wh = np.asarray(Wh, np.float32)
    Wattn = np.asarray(Wattn, np.float32)
    b = np.asarray(b, np.float32)

    W_comb = np.concatenate([Wh, Wattn], axis=0)
    h0 = A_flat.mean(axis=2).astype(np.float32)
    scores0 = (np.einsum('nh,nhl->nl', h0, A_flat) * SCALE).astype(np.float32)
    xT = np.ascontiguousarray(
        x.transpose(2, 0, 1).reshape(4, 128, N, T)).astype(BF16)
    h0T = np.ascontiguousarray(
        h0.T.reshape(8, 128, N).transpose(1, 0, 2)).astype(BF16)
    eyes = (np.eye(N, dtype=np.float32) * SCALE)

    in_maps = []
    for j in range(R):
        cols = np.array([g * H + j * HS + i for g in range(4) for i in range(HS)])
        hsl = slice(j * HS, (j + 1) * HS)
        in_maps.append({
            "xT": xT,
            "whaj": np.ascontiguousarray(
                W_comb[:, cols].reshape(16, 128, GS)).astype(BF16),
            "wxj": np.ascontiguousarray(
                Wx[:, cols].reshape(4, 128, GS)).astype(BF16),
            "brep": np.tile(b[cols], (128, 1)).astype(np.float32),
            "asT": np.ascontiguousarray(
                A_flat[:, hsl, :].transpose(1, 2, 0)).astype(BF16),
            "anm": np.ascontiguousarray(A_flat[:, hsl, :]),
            "eyes": eyes,
            "iden": np.eye(128, dtype=np.float32),
            "h0T": h0T,
            "c0": np.ascontiguousarray(h0[:, hsl]),
            "s0": scores0,
        })
    return in_maps


def _build():
    nc = bass.Bass("TRN2", target_bir_lowering=False, debug=False, num_devices=R)
    rg = [list(range(R))]

    xT_d = nc.dram_tensor("xT", [4, 128, N, T], BF, kind="ExternalInput")
    whaj_d = nc.dram_tensor("whaj", [16, 128, GS], BF, kind="ExternalInput")
    wxj_d = nc.dram_tensor("wxj", [4, 128, GS], BF, kind="ExternalInput")
    brep_d = nc.dram_tensor("brep", [128, GS], F32, kind="ExternalInput")
    asT_d = nc.dram_tensor("asT", [128, L, N], BF, kind="ExternalInput")
    anm_d = nc.dram_tensor("anm", [N, HS, L], F32, kind="ExternalInput")
    eyes_d = nc.dram_tensor("eyes", [N, N], F32, kind="ExternalInput")
    iden_d = nc.dram_tensor("iden", [128, 128], F32, kind="ExternalInput")
    h0T_d = nc.dram_tensor("h0T", [128, 8, N], BF, kind="ExternalInput")
    c0_d = nc.dram_tensor("c0", [N, HS], F32, kind="ExternalInput")
    s0_d = nc.dram_tensor("s0", [N, L], F32, kind="ExternalInput")
    out_d = nc.dram_tensor("out", [N, T, HS], F32, kind="ExternalOutput")

    with tile.TileContext(nc) as tc:
        with tc.tile_pool(name="const", bufs=1) as cp, \
             tc.tile_pool(name="state", bufs=1) as st, \
             tc.tile_pool(name="dramx", bufs=1, space="DRAM") as dpx, \
             tc.tile_pool(name="dram", bufs=2, space="DRAM") as dp:

            xwxb = dpx.tile([N, T, GS], F32, name="xwxb")

            whaj = cp.tile([128, 16, GS], BF, name="whaj")
            asT = cp.tile([128, L, N], BF, name="asT")
            anm = cp.tile([N, HS, L], F32, name="anm")
            eyes = cp.tile([N, N], F32, name="eyes")
            iden = cp.tile([128, 128], F32, name="iden")
            nc.sync.dma_start(out=whaj[:, :, :], in_=whaj_d.rearrange("k p g -> p k g"))
            nc.sync.dma_start(out=asT[:, :, :], in_=asT_d[:, :, :])
            nc.sync.dma_start(out=anm[:, :, :], in_=anm_d[:, :, :])
            nc.sync.dma_start(out=eyes[:, :], in_=eyes_d[:, :])
            nc.sync.dma_start(out=iden[:, :], in_=iden_d[:, :])

            c = st.tile([N, HS], F32, name="c")
            nc.sync.dma_start(out=c[:, :], in_=c0_d[:, :])

            # phase 0: xwxb[n, t, :] = x[n, t] @ Wx_j + b_j
            with tc.tile_pool(name="ph0sb", bufs=2) as p0s, \
                 tc.tile_pool(name="ph0ps", bufs=8, space="PSUM") as p0p:
                wxj = p0s.tile([128, 4, GS], BF, name="wxj", tag="wxc", bufs=1)
                brep = p0s.tile([128, GS], F32, name="brep", tag="wxc2", bufs=1)
                nc.sync.dma_start(out=wxj[:, :, :],
                                  in_=wxj_d.rearrange("k p g -> p k g"))
                nc.sync.dma_start(out=brep[:, :], in_=brep_d[:, :])
                for g in range(8):
                    pss = [p0p.tile([T, GS], F32, name=f"ps{g}_{i}", tag="p0ps")
                           for i in range(8)]
                    for kt in range(4):
                        xg = p0s.tile([128, 8, T], BF, name="xg", tag="xg")
                        nc.sync.dma_start(
                            out=xg[:, :, :],
                            in_=xT_d[kt, :, g * 8:(g + 1) * 8, 0:T])
                        for i in range(8):
                            nc.tensor.matmul(pss[i][:, :], xg[:, i, :],
                                             wxj[:, kt, :],
                                             start=(kt == 0), stop=(kt == 3))
                    for i in range(8):
                        sg = p0s.tile([T, GS], F32, name="sg", tag="sg")
                        nc.vector.tensor_add(out=sg[:, :], in0=pss[i][:, :],
                                             in1=brep[0:T, :])
                        nc.sync.dma_start(out=xwxb[g * 8 + i, 0:T, :], in_=sg[:, :])

            with tc.tile_pool(name="wk", bufs=2) as wk, \
                 tc.tile_pool(name="ps_a", bufs=2, space="PSUM") as ps_a, \
                 tc.tile_pool(name="ps_x", bufs=2, space="PSUM") as ps_x, \
                 tc.tile_pool(name="ps_t", bufs=1, space="PSUM") as ps_t:

                hT_full = wk.tile([128, 8, N], BF, name="hT0", tag="hT_full")
                nc.sync.dma_start(out=hT_full[:, :, :], in_=h0T_d[:, :, :])
                scores = wk.tile([N, L], F32, name="scores0", tag="scores")
                nc.sync.dma_start(out=scores[:, :], in_=s0_d[:, :])

                for t in range(T):
                    nm = wk.tile([N, 1], F32, name="nm", tag="nm")
                    nc.vector.reduce_max(out=nm[:, :], in_=scores[:, :], axis=AX.X,
                                         negate=True)
                    e = wk.tile([N, L], F32, name="e", tag="e")
                    nc.scalar.activation(e[:, :], scores[:, :], AF.Exp,
                                         bias=nm[:, 0:1])
                    se = wk.tile([N, 1], F32, name="se", tag="se")
                    nc.vector.reduce_sum(out=se[:, :], in_=e[:, :], axis=AX.X)
                    rse = wk.tile([N, 1], F32, name="rse", tag="rse")
                    nc.vector.reciprocal(out=rse[:, :], in_=se[:, :])
                    w = wk.tile([N, L], F32, name="w", tag="w")
                    nc.vector.tensor_scalar(out=w[:, :], in0=e[:, :],
                                            scalar1=rse[:, 0:1], scalar2=None,
                                            op0=AluOpType.mult)
                    atm = wk.tile([N, HS, L], F32, name="atm", tag="atm")
                    nc.vector.tensor_tensor(
                        out=atm[:, :, :], in0=anm[:, :, :],
                        in1=w[:, None, :].broadcast_to((N, HS, L)),
                        op=AluOpType.mult)
                    attn_j = wk.tile([N, HS], F32, name="attn_j", tag="attn_j")
                    nc.vector.reduce_sum(out=attn_j[:, :], in_=atm[:, :, :],
                                         axis=AX.X)
                    pt1 = ps_t.tile([128, N], F32, name="pt1", tag="pt1")
                    nc.tensor.transpose(pt1[:, :], attn_j[:, :], iden[0:N, 0:N])
                    attnT_bf = wk.tile([128, N], BF, name="attnT_bf", tag="attnT_bf")
                    nc.vector.tensor_copy(out=attnT_bf[:, :], in_=pt1[:, :])
                    sendB = dp.tile([128, N], BF, name="sendB", tag="sendB")
                    recvB = dp.tile([R, 128, N], BF, name="recvB", tag="recvB",
                                    addr_space="Shared")
                    nc.sync.dma_start(out=sendB[:, :], in_=attnT_bf[:, :])
                    nc.gpsimd.collective_compute(
                        "AllGather", AluOpType.bypass, replica_groups=rg,
                        ins=[sendB[:, :].opt()], outs=[recvB[:, :, :].opt()])
                    attnT_full = wk.tile([128, 8, N], BF, name="attnT_full",
                                         tag="attnT_full")
                    nc.sync.dma_start(out=attnT_full[:, :, :],
                                      in_=recvB.rearrange("r p n -> p r n"))
                    pa = ps_a.tile([N, GS], F32, name="pa", tag="pa")
                    for kt in range(8):
                        nc.tensor.matmul(pa[:, :], hT_full[:, kt, :],
                                         whaj[:, kt, :],
                                         start=(kt == 0), stop=False)
                    for kt in range(8):
                        nc.tensor.matmul(pa[:, :], attnT_full[:, kt, :],
                                         whaj[:, 8 + kt, :],
                                         start=False, stop=(kt == 7))
                    xwx_t = wk.tile([N, GS], F32, name="xwx_t", tag="xwx_t")
                    nc.sync.dma_start(out=xwx_t[:, :], in_=xwxb[:, t, :])
                    s_g = wk.tile([N, GS], F32, name="s_g", tag="s_g")
                    nc.vector.tensor_add(out=s_g[:, :], in0=pa[:, :],
                                         in1=xwx_t[:, :])
                    sig = wk.tile([N, 3 * HS], F32, name="sig", tag="sig")
                    nc.scalar.activation(sig[:, :], s_g[:, 0:3 * HS], AF.Sigmoid)
                    gt = wk.tile([N, HS], F32, name="gt", tag="gt")
                    nc.scalar.activation(gt[:, :], s_g[:, 3 * HS:4 * HS], AF.Tanh)
                    t1 = wk.tile([N, HS], F32, name="t1", tag="t1")
                    nc.vector.tensor_mul(out=t1[:, :], in0=sig[:, 0:HS],
                                         in1=gt[:, :])
                    nc.vector.tensor_mul(out=c[:, :], in0=sig[:, HS:2 * HS],
                                         in1=c[:, :])
                    nc.vector.tensor_add(out=c[:, :], in0=c[:, :], in1=t1[:, :])
                    tanc = wk.tile([N, HS], F32, name="tanc", tag="tanc")
                    nc.scalar.activation(tanc[:, :], c[:, :], AF.Tanh)
                    h_j = wk.tile([N, HS], F32, name="h_j", tag="h_j")
                    nc.vector.tensor_mul(out=h_j[:, :], in0=sig[:, 2 * HS:3 * HS],
                                         in1=tanc[:, :])
                    nc.sync.dma_start(out=out_d[:, t, :], in_=h_j[:, :])
                    if t == T - 1:
                        break
                    pt2 = ps_t.tile([128, N], F32, name="pt2", tag="pt2")
                    nc.tensor.transpose(pt2[:, :], h_j[:, :], iden[0:N, 0:N])
                    hT_bf = wk.tile([128, N], BF, name="hT_bf", tag="hT_bf")
                    nc.vector.tensor_copy(out=hT_bf[:, :], in_=pt2[:, :])
                    px = ps_x.tile([N, L, N], F32, name="px", tag="px")
                    nc.tensor.matmul(px[:, 0:8, :], hT_bf[:, :], asT[:, 0:8, :],
                                     start=True, stop=True)
                    nc.tensor.matmul(px[:, 8:16, :], hT_bf[:, :], asT[:, 8:16, :],
                                     start=True, stop=True)
                    msk = wk.tile([N, L, N], F32, name="msk", tag="msk")
                    nc.vector.tensor_tensor(
                        out=msk[:, :, :], in0=px[:, :, :],
                        in1=eyes[:, None, :].broadcast_to((N, L, N)),
                        op=AluOpType.mult)
                    spart = wk.tile([N, L], F32, name="spart", tag="spart")
                    nc.vector.reduce_sum(out=spart[:, :], in_=msk[:, :, :],
                                         axis=AX.X)
                    spart_bf = wk.tile([N, L], BF, name="spart_bf", tag="spart_bf")
                    nc.vector.tensor_copy(out=spart_bf[:, :], in_=spart[:, :])
                    sendA = dp.tile([9216], BF, name="sendA", tag="sendA")
                    recvA = dp.tile([R, 9216], BF, name="recvA", tag="recvA",
                                    addr_space="Shared")
                    nc.sync.dma_start(
                        out=sendA[0:8192].rearrange("(p n) -> p n", p=128),
                        in_=hT_bf[:, :])
                    nc.sync.dma_start(
                        out=sendA[8192:9216].rearrange("(n l) -> n l", n=N),
                        in_=spart_bf[:, :])
                    nc.gpsimd.collective_compute(
                        "AllGather", AluOpType.bypass, replica_groups=rg,
                        ins=[sendA[:].opt()], outs=[recvA[:, :].opt()])
                    hT_full = wk.tile([128, 8, N], BF, name="hT_full",
                                      tag="hT_full")
                    nc.sync.dma_start(
                        out=hT_full[:, :, :],
                        in_=recvA[:, 0:8192].rearrange("r (p n) -> p r n", p=128))
                    sparts = wk.tile([N, 8, L], BF, name="sparts", tag="sparts")
                    nc.sync.dma_start(
                        out=sparts[:, :, :],
                        in_=recvA[:, 8192:9216].rearrange("r (n l) -> n r l", n=N))
                    s4 = wk.tile([N, 4, L], F32, name="s4", tag="s4")
                    nc.vector.tensor_add(out=s4[:, :, :], in0=sparts[:, 0:4, :],
                                         in1=sparts[:, 4:8, :])
                    s2 = wk.tile([N, 2, L], F32, name="s2", tag="s2")
                    nc.vector.tensor_add(out=s2[:, :, :], in0=s4[:, 0:2, :],
                                         in1=s4[:, 2:4, :])
                    scores = wk.tile([N, L], F32, name="scores", tag="scores")
                    nc.vector.tensor_add(out=scores[:, :], in0=s2[:, 0, :],
                                         in1=s2[:, 1, :])

    _split_waits(nc, cap=1)
    return nc


_NC_CACHE = None


def kernel(**inputs) -> np.ndarray:
    global _NC_CACHE
    in_maps = _prep_inputs(**inputs)
    if _NC_CACHE is None:
        _NC_CACHE = _build()
    res = run_bass_kernel_spmd(_NC_CACHE, in_maps, core_ids=list(range(R)))
    out = np.zeros((N, T, H), dtype=np.float32)
    for j, r in enumerate(res.results):
        out[:, :, j * HS:(j + 1) * HS] = np.asarray(r["out"]).reshape(N, T, HS)
    return out


# revision 4
# speedup vs baseline: 1.0050x; 1.0050x over previous
"""Trainium2 Bass kernel for nn_CaptioningRNN (attention-LSTM over T=128 steps).

Sharding: tensor-parallel over the 4H gate dimension across 8 NeuronCores.
Core j owns H-slice j (128 h-rows) of each of the 4 gates (gate columns
{g*1024 + j*128 + i}), so the per-step LSTM cell state (c, h) for that
slice lives entirely on core j. Per step:
  - replicated softmax over the 16 attention scores,
  - local attention slice (DVE mul+reduce against the A slice),
  - AllGather of the bf16 attn^T slices,
  - 16-ktile PE matmul [h; attn] @ [Wh; Wattn] gate-slice (bf16, f32 psum),
  - gates + c/h update (ACT/DVE, f32),
  - PE transpose of h, scores-cross on PE + masked diagonal reduce,
  - AllGather of (hT slice | score partials) in bf16.
x @ Wx + b is precomputed on-device into DRAM in a parallel phase 0.
Host side does layout-only prep (slicing/transposes/casts) and the final
output assembly out[:, :, Hj] <- core j.
"""
import numpy as np
import ml_dtypes

import bass_rust
import concourse.bass as bass
import concourse.mybir as mybir
from concourse import tile
from concourse.alu_op_type import AluOpType
from concourse.bass_utils import run_bass_kernel_spmd

BF16 = ml_dtypes.bfloat16
F32 = mybir.dt.float32
BF = mybir.dt.bfloat16
AF = mybir.ActivationFunctionType
AX = mybir.AxisListType

N, T, D, H, L, R = 64, 128, 512, 1024, 16, 8
HS, GS = H // R, 4 * H // R  # 128, 512
SCALE = 1.0 / np.sqrt(H)


def _split_waits(nc, cap=1):
    """Walrus here rejects >cap sync waits per instruction; hoist extras
    onto preceding same-engine NOPs."""
    ctr = 0
    for fn in nc.m.functions:
        for bb in fn.blocks:
            out, changed = [], False
            for ins in bb.instructions:
                si = ins.sync_info
                if si is not None and si.on_wait and len(si.on_wait) > cap:
                    waits = list(si.on_wait)
                    extra, keep = waits[:-cap], waits[-cap:]
                    for i in range(0, len(extra), cap):
                        out.append(bass_rust.InstNoOp(
                            name=f"zz_waitsplit_{ctr}", engine=ins.engine,
                            sync_info=bass_rust.SyncInfo(
                                on_wait=extra[i:i + cap], on_update=[])))
                        ctr += 1
                    ins.sync_info = bass_rust.SyncInfo(
                        on_wait=keep, on_update=list(si.on_update or []))
                    changed = True
                out.append(ins)
            if changed:
                bb.instructions = out
    return ctr


def _prep_inputs(x, A, Wx, Wh, Wattn, b):
    x = np.asarray(x, np.float32)
    A_flat = np.asarray(A, np.float32).reshape(N, H, L)
    Wx = np.asarray(Wx, np.float32)
    Wh = np.asarray(Wh, np.float32)
    Wattn = np.asarray(Wattn, np.float32)
    b = np.asarray(b, np.float32)

    W_comb = np.concatenate([Wh, Wattn], axis=0)
    h0 = A_flat.mean(axis=2).astype(np.float32)
    scores0 = (np.einsum('nh,nhl->nl', h0, A_flat) * SCALE).astype(np.float32)
    xT = np.ascontiguousarray(
        x.transpose(2, 0, 1).reshape(4, 128, N, T)).astype(BF16)
    h0T = np.ascontiguousarray(
        h0.T.reshape(8, 128, N).transpose(1, 0, 2)).astype(BF16)
    eyes = (np.eye(N, dtype=np.float32) * SCALE)

    in_maps = []
    for j in range(R):
        cols = np.array([g * H + j * HS + i for g in range(4) for i in range(HS)])
        hsl = slice(j * HS, (j + 1) * HS)
        in_maps.append({
            "xT": xT,
            "whaj": np.ascontiguousarray(
                W_comb[:, cols].reshape(16, 128, GS)).astype(BF16),
            "wxj": np.ascontiguousarray(
                Wx[:, cols].reshape(4, 128, GS)).astype(BF16),
            "brep": np.tile(b[cols], (128, 1)).astype(np.float32),
            "asT": np.ascontiguousarray(
                A_flat[:, hsl, :].transpose(1, 2, 0)).astype(BF16),
            "anm": np.ascontiguousarray(A_flat[:, hsl, :]).astype(BF16),
            "eyes": eyes,
            "iden": np.eye(128, dtype=np.float32),
            "h0T": h0T,
            "c0": np.ascontiguousarray(h0[:, hsl]),
            "s0": scores0,
        })
    return in_maps


def _build():
    nc = bass.Bass("TRN2", target_bir_lowering=False, debug=False, num_devices=R)
    rg = [list(range(R))]

    xT_d = nc.dram_tensor("xT", [4, 128, N, T], BF, kind="ExternalInput")
    whaj_d = nc.dram_tensor("whaj", [16, 128, GS], BF, kind="ExternalInput")
    wxj_d = nc.dram_tensor("wxj", [4, 128, GS], BF, kind="ExternalInput")
    brep_d = nc.dram_tensor("brep", [128, GS], F32, kind="ExternalInput")
    asT_d = nc.dram_tensor("asT", [128, L, N], BF, kind="ExternalInput")
    anm_d = nc.dram_tensor("anm", [N, HS, L], BF, kind="ExternalInput")
    eyes_d = nc.dram_tensor("eyes", [N, N], F32, kind="ExternalInput")
    iden_d = nc.dram_tensor("iden", [128, 128], F32, kind="ExternalInput")
    h0T_d = nc.dram_tensor("h0T", [128, 8, N], BF, kind="ExternalInput")
    c0_d = nc.dram_tensor("c0", [N, HS], F32, kind="ExternalInput")
    s0_d = nc.dram_tensor("s0", [N, L], F32, kind="ExternalInput")
    out_d = nc.dram_tensor("out", [N, T, HS], F32, kind="ExternalOutput")

    with tile.TileContext(nc) as tc:
        with tc.tile_pool(name="const", bufs=1) as cp, \
             tc.tile_pool(name="state", bufs=1) as st, \
             tc.tile_pool(name="dramx", bufs=1, space="DRAM") as dpx, \
             tc.tile_pool(name="dram", bufs=2, space="DRAM") as dp:

            xwxb = dpx.tile([N, T, GS], F32, name="xwxb")

            whaj = cp.tile([128, 16, GS], BF, name="whaj")
            asT = cp.tile([128, L, N], BF, name="asT")
            anm = cp.tile([N, HS, L], BF, name="anm")
            eyes = cp.tile([N, N], F32, name="eyes")
            iden = cp.tile([128, 128], F32, name="iden")
            nc.sync.dma_start(out=whaj[:, :, :], in_=whaj_d.rearrange("k p g -> p k g"))
            nc.sync.dma_start(out=asT[:, :, :], in_=asT_d[:, :, :])
            nc.sync.dma_start(out=anm[:, :, :], in_=anm_d[:, :, :])
            nc.sync.dma_start(out=eyes[:, :], in_=eyes_d[:, :])
            nc.sync.dma_start(out=iden[:, :], in_=iden_d[:, :])

            c = st.tile([N, HS], F32, name="c")
            nc.sync.dma_start(out=c[:, :], in_=c0_d[:, :])

            # phase 0: xwxb[n, t, :] = x[n, t] @ Wx_j + b_j
            with tc.tile_pool(name="ph0sb", bufs=2) as p0s, \
                 tc.tile_pool(name="ph0ps", bufs=8, space="PSUM") as p0p:
                wxj = p0s.tile([128, 4, GS], BF, name="wxj", tag="wxc", bufs=1)
                brep = p0s.tile([128, GS], F32, name="brep", tag="wxc2", bufs=1)
                nc.sync.dma_start(out=wxj[:, :, :],
                                  in_=wxj_d.rearrange("k p g -> p k g"))
                nc.sync.dma_start(out=brep[:, :], in_=brep_d[:, :])
                for g in range(8):
                    pss = [p0p.tile([T, GS], F32, name=f"ps{g}_{i}", tag="p0ps")
                           for i in range(8)]
                    for kt in range(4):
                        xg = p0s.tile([128, 8, T], BF, name="xg", tag="xg")
                        nc.sync.dma_start(
                            out=xg[:, :, :],
                            in_=xT_d[kt, :, g * 8:(g + 1) * 8, 0:T])
                        for i in range(8):
                            nc.tensor.matmul(pss[i][:, :], xg[:, i, :],
                                             wxj[:, kt, :],
                                             start=(kt == 0), stop=(kt == 3))
                    for i in range(8):
                        sg = p0s.tile([T, GS], F32, name="sg", tag="sg")
                        nc.vector.tensor_add(out=sg[:, :], in0=pss[i][:, :],
                                             in1=brep[0:T, :])
                        nc.sync.dma_start(out=xwxb[g * 8 + i, 0:T, :], in_=sg[:, :])

            with tc.tile_pool(name="wk", bufs=2) as wk, \
                 tc.tile_pool(name="ps_a", bufs=2, space="PSUM") as ps_a, \
                 tc.tile_pool(name="ps_x", bufs=2, space="PSUM") as ps_x, \
                 tc.tile_pool(name="ps_t", bufs=1, space="PSUM") as ps_t:

                hT_full = wk.tile([128, 8, N], BF, name="hT0", tag="hT_full")
                nc.sync.dma_start(out=hT_full[:, :, :], in_=h0T_d[:, :, :])
                scores = wk.tile([N, L], F32, name="scores0", tag="scores")
                nc.sync.dma_start(out=scores[:, :], in_=s0_d[:, :])

                for t in range(T):
                    # softmax without max-sub (scores bounded); exp/sigmoid via
                    # tanh so ScalarE keeps ONE activation table loaded:
                    # exp(x) = (1 + tanh(x/2)) / (1 - tanh(x/2))
                    th = wk.tile([N, L], F32, name="th", tag="th")
                    nc.scalar.activation(th[:, :], scores[:, :], AF.Tanh, scale=0.5)
                    den = wk.tile([N, L], F32, name="den", tag="den")
                    nc.vector.tensor_scalar(out=den[:, :], in0=th[:, :],
                                            scalar1=-1.0, scalar2=1.0,
                                            op0=AluOpType.mult, op1=AluOpType.add)
                    rden = wk.tile([N, L], F32, name="rden", tag="rden")
                    nc.vector.reciprocal(out=rden[:, :], in_=den[:, :])
                    num = wk.tile([N, L], F32, name="num", tag="num")
                    nc.vector.tensor_scalar(out=num[:, :], in0=th[:, :],
                                            scalar1=1.0, scalar2=None,
                                            op0=AluOpType.add)
                    e = wk.tile([N, L], F32, name="e", tag="e")
                    nc.vector.tensor_mul(out=e[:, :], in0=num[:, :], in1=rden[:, :])
                    se = wk.tile([N, 1], F32, name="se", tag="se")
                    nc.vector.reduce_sum(out=se[:, :], in_=e[:, :], axis=AX.X)
                    rse = wk.tile([N, 1], F32, name="rse", tag="rse")
                    nc.vector.reciprocal(out=rse[:, :], in_=se[:, :])
                    w = wk.tile([N, L], F32, name="w", tag="w")
                    nc.vector.tensor_scalar(out=w[:, :], in0=e[:, :],
                                            scalar1=rse[:, 0:1], scalar2=None,
                                            op0=AluOpType.mult)
                    w_bf = wk.tile([N, L], BF, name="w_bf", tag="w_bf")
                    nc.vector.tensor_copy(out=w_bf[:, :], in_=w[:, :])
                    atm = wk.tile([N, HS, L], BF, name="atm", tag="atm")
                    nc.vector.tensor_tensor(
                        out=atm[:, :, :], in0=anm[:, :, :],
                        in1=w_bf[:, None, :].broadcast_to((N, HS, L)),
                        op=AluOpType.mult)
                    attn_j = wk.tile([N, HS], F32, name="attn_j", tag="attn_j")
                    nc.vector.reduce_sum(out=attn_j[:, :], in_=atm[:, :, :],
                                         axis=AX.X)
                    pt1 = ps_t.tile([128, N], F32, name="pt1", tag="pt1")
                    nc.tensor.transpose(pt1[:, :], attn_j[:, :], iden[0:N, 0:N])
                    attnT_bf = wk.tile([128, N], BF, name="attnT_bf", tag="attnT_bf")
                    nc.vector.tensor_copy(out=attnT_bf[:, :], in_=pt1[:, :])
                    sendB = dp.tile([128, N], BF, name="sendB", tag="sendB")
                    recvB = dp.tile([R, 128, N], BF, name="recvB", tag="recvB",
                                    addr_space="Shared")
                    nc.sync.dma_start(out=sendB[:, :], in_=attnT_bf[:, :])
                    nc.gpsimd.collective_compute(
                        "AllGather", AluOpType.bypass, replica_groups=rg,
                        ins=[sendB[:, :].opt()], outs=[recvB[:, :, :].opt()])
                    attnT_full = wk.tile([128, 8, N], BF, name="attnT_full",
                                         tag="attnT_full")
                    for r_ in range(R):
                        nc.sync.dma_start(out=attnT_full[:, r_, :],
                                          in_=recvB[r_, :, :])
                    pa = ps_a.tile([N, GS], F32, name="pa", tag="pa")
                    for kt in range(8):
                        nc.tensor.matmul(pa[:, :], hT_full[:, kt, :],
                                         whaj[:, kt, :],
                                         start=(kt == 0), stop=False)
                    for kt in range(8):
                        nc.tensor.matmul(pa[:, :], attnT_full[:, kt, :],
                                         whaj[:, 8 + kt, :],
                                         start=False, stop=(kt == 7))
                    xwx_t = wk.tile([N, GS], F32, name="xwx_t", tag="xwx_t")
                    nc.sync.dma_start(out=xwx_t[:, :], in_=xwxb[:, t, :])
                    s_g = wk.tile([N, GS], F32, name="s_g", tag="s_g")
                    nc.vector.tensor_add(out=s_g[:, :], in0=pa[:, :],
                                         in1=xwx_t[:, :])
                    sg3 = wk.tile([N, 3 * HS], F32, name="sg3", tag="sg3")
                    nc.scalar.activation(sg3[:, :], s_g[:, 0:3 * HS], AF.Tanh,
                                         scale=0.5)
                    sig = wk.tile([N, 3 * HS], F32, name="sig", tag="sig")
                    nc.vector.tensor_scalar(out=sig[:, :], in0=sg3[:, :],
                                            scalar1=1.0, scalar2=0.5,
                                            op0=AluOpType.add, op1=AluOpType.mult)
                    gt = wk.tile([N, HS], F32, name="gt", tag="gt")
                    nc.scalar.activation(gt[:, :], s_g[:, 3 * HS:4 * HS], AF.Tanh)
                    t1 = wk.tile([N, HS], F32, name="t1", tag="t1")
                    nc.vector.tensor_mul(out=t1[:, :], in0=sig[:, 0:HS],
                                         in1=gt[:, :])
                    nc.vector.tensor_mul(out=c[:, :], in0=sig[:, HS:2 * HS],
                                         in1=c[:, :])
                    nc.vector.tensor_add(out=c[:, :], in0=c[:, :], in1=t1[:, :])
                    tanc = wk.tile([N, HS], F32, name="tanc", tag="tanc")
                    nc.scalar.activation(tanc[:, :], c[:, :], AF.Tanh)
                    h_j = wk.tile([N, HS], F32, name="h_j", tag="h_j")
                    nc.vector.tensor_mul(out=h_j[:, :], in0=sig[:, 2 * HS:3 * HS],
                                         in1=tanc[:, :])
                    nc.sync.dma_start(out=out_d[:, t, :], in_=h_j[:, :])
                    if t == T - 1:
                        break
                    pt2 = ps_t.tile([128, N], F32, name="pt2", tag="pt2")
                    nc.tensor.transpose(pt2[:, :], h_j[:, :], iden[0:N, 0:N])
                    hT_bf = wk.tile([128, N], BF, name="hT_bf", tag="hT_bf")
                    nc.vector.tensor_copy(out=hT_bf[:, :], in_=pt2[:, :])
                    px = ps_x.tile([N, L, N], F32, name="px", tag="px")
                    nc.tensor.matmul(px[:, 0:8, :], hT_bf[:, :], asT[:, 0:8, :],
                                     start=True, stop=True)
                    nc.tensor.matmul(px[:, 8:16, :], hT_bf[:, :], asT[:, 8:16, :],
                                     start=True, stop=True)
                    msk = wk.tile([N, L, N], F32, name="msk", tag="msk")
                    nc.vector.tensor_tensor(
                        out=msk[:, :, :], in0=px[:, :, :],
                        in1=eyes[:, None, :].broadcast_to((N, L, N)),
                        op=AluOpType.mult)
                    spart = wk.tile([N, L], F32, name="spart", tag="spart")
                    nc.vector.reduce_sum(out=spart[:, :], in_=msk[:, :, :],
                                         axis=AX.X)
                    spart_bf = wk.tile([N, L], BF, name="spart_bf", tag="spart_bf")
                    nc.vector.tensor_copy(out=spart_bf[:, :], in_=spart[:, :])
                    sendA = dp.tile([9216], BF, name="sendA", tag="sendA")
                    recvA = dp.tile([R, 9216], BF, name="recvA", tag="recvA",
                                    addr_space="Shared")
                    nc.sync.dma_start(
                        out=sendA[0:8192].rearrange("(p n) -> p n", p=128),
                        in_=hT_bf[:, :])
                    nc.sync.dma_start(
                        out=sendA[8192:9216].rearrange("(n l) -> n l", n=N),
                        in_=spart_bf[:, :])
                    nc.gpsimd.collective_compute(
                        "AllGather", AluOpType.bypass, replica_groups=rg,
                        ins=[sendA[:].opt()], outs=[recvA[:, :].opt()])
                    hT_full = wk.tile([128, 8, N], BF, name="hT_full",
                                      tag="hT_full")
                    for r_ in range(R):
                        nc.sync.dma_start(
                            out=hT_full[:, r_, :],
                            in_=recvA[r_, 0:8192].rearrange("(p n) -> p n", p=128))
                    sparts = wk.tile([N, 8, L], BF, name="sparts", tag="sparts")
                    nc.sync.dma_start(
                        out=sparts[:, :, :],
                        in_=recvA[:, 8192:9216].rearrange("r (n l) -> n r l", n=N))
                    s4 = wk.tile([N, 4, L], F32, name="s4", tag="s4")
                    nc.vector.tensor_add(out=s4[:, :, :], in0=sparts[:, 0:4, :],
                                         in1=sparts[:, 4:8, :])
                    s2 = wk.tile([N, 2, L], F32, name="s2", tag="s2")
                    nc.vector.tensor_add(out=s2[:, :, :], in0=s4[:, 0:2, :],
                                         in1=s4[:, 2:4, :])
                    scores = wk.tile([N, L], F32, name="scores", tag="scores")
                    nc.vector.tensor_add(out=scores[:, :], in0=s2[:, 0, :],
                                         in1=s2[:, 1, :])

    _split_waits(nc, cap=1)
    return nc


_NC_CACHE = None


def kernel(**inputs) -> np.ndarray:
    global _NC_CACHE
    in_maps = _prep_inputs(**inputs)
    if _NC_CACHE is None:
        _NC_CACHE = _build()
    res = run_bass_kernel_spmd(_NC_CACHE, in_maps, core_ids=list(range(R)))
    out = np.zeros((N, T, H), dtype=np.float32)
    for j, r in enumerate(res.results):
        out[:, :, j * HS:(j + 1) * HS] = np.asarray(r["out"]).reshape(N, T, HS)
    return out
